# revision 24
# baseline (speedup 1.0000x reference)
"""Trainium2 Bass kernel: depthwise (per-sample, per-channel) 15x15 'same'
true convolution of 1024x3 images of 128x128, data-parallel over 8 NeuronCores.

Formulation (per (bn,c) "job", P=128, K=15, pad=7):
    out[y,x] = sum_{dy,dx} Xpad[y+dy, x+dx] * W[dy,dx],   W = flip(kernel)
The 128 output rows are split into 4 blocks of 32. Per dx the contribution of
all 4 blocks is ONE banded-Toeplitz matmul
    T_dx[i, j] = W[i-j, dx]  (i in 0..45, j in 0..31, band 0 <= i-j < 15)
    out[32b+j, x] += sum_i T_dx[i, j] * Xpad[32b+i, x+dx]
i.e. [K=46, M=32, N=4*128] accumulated over the 15 dx values in PSUM.
The PE array runs as 8 concurrent 64x32 tiles (tile_position): tile (R, c)
streams from SBUF partitions R..R+45 and writes PSUM partitions c..c+31 of
bank R/64. One "set" = 8 jobs in flight, 15 matmuls per job. Operands are
fp16 (PSUM accumulates fp32), output fp16. Host prepares the SBUF-layout
operands; sharding is pure data parallel over BN (128 samples x 3 channels
= 384 jobs = 48 sets per core).
"""
import sys

sys.path.insert(0, "/opt/trn_rl_repo")

import numpy as np

_N_CORES = 8
_BN, _C, _P, _K = 1024, 3, 128, 15
_PAIRS_PER_CORE = (_BN // _N_CORES) * _C  # 384
_SETS_PER_CORE = _PAIRS_PER_CORE // 8  # 48

_nc_cache = {}


def _build_nc(n_sets: int):
    import concourse.bacc as bacc
    import concourse.mybir as mybir
    from concourse import tile

    FP16 = mybir.dt.float16
    FP32 = mybir.dt.float32

    nc = bacc.Bacc("TRN2", target_bir_lowering=False, debug=False)
    xprep_d = nc.dram_tensor("xprep", [n_sets, 128, 4, 4, 142], FP16, kind="ExternalInput")
    tprep_d = nc.dram_tensor("tprep", [n_sets, 128, 4, 15, 32], FP16, kind="ExternalInput")
    out_d = nc.dram_tensor("out", [n_sets, 128, 2, 4, 128], FP16, kind="ExternalOutput")

    with tile.TileContext(nc) as tc:
        with (
            tc.tile_pool(name="xp", bufs=4) as x_pool,
            tc.tile_pool(name="tp", bufs=4) as t_pool,
            tc.tile_pool(name="op", bufs=3) as o_pool,
            tc.tile_pool(name="ps", bufs=2, space="PSUM") as ps_pool,
            tc.tile_pool(name="wps", bufs=1, space="PSUM") as wps_pool,
        ):
            # HAM warmup: 64-row-mode (h-group) matmuls never trip the PE
            # activity monitor's un-throttle, so the whole kernel would run
            # at 1.2GHz. Fire it with a >3.4us continuous burst of 32-row-
            # mode matmuls on scratch data first; the h-mode stream then
            # keeps it warm.
            wsb = x_pool.tile([128, 1024], FP16, tag="wsb", bufs=1)
            nc.vector.memset(wsb[:], 1.0)
            wps = [
                wps_pool.tile([128, 512], FP32, tag=f"wps{r}", name=f"wps{r}")
                for r in range(4)
            ]
            for i in range(40):
                for r in range(4):
                    for c in range(4):
                        nc.tensor.matmul(
                            wps[r][32 * c:32 * c + 16, :],
                            wsb[32 * r:32 * r + 30, 0:16],
                            wsb[32 * r:32 * r + 30, 16:528],
                            start=(i == 0), stop=(i == 39),
                            tile_position=(32 * r, 32 * c),
                        )
            for s in range(n_sets):
                xt = x_pool.tile([128, 4, 4, 142], FP16, tag="xt")
                tt = t_pool.tile([128, 4, 15, 32], FP16, tag="tt")
                nc.sync.dma_start(out=xt[:], in_=xprep_d[s])
                nc.scalar.dma_start(out=tt[:], in_=tprep_d[s])

                ps = [
                    ps_pool.tile([128, 4, 128], FP32, tag=f"ps{R}", name=f"ps{R}")
                    for R in range(2)
                ]

                for dx in range(15):
                    for R in range(2):
                        for cg in range(4):
                            nc.tensor.matmul(
                                ps[R][32 * cg:32 * cg + 32, :, :],
                                tt[64 * R:64 * R + 46, cg, dx, :],
                                xt[64 * R:64 * R + 46, cg, :, dx:dx + 128],
                                start=(dx == 0), stop=(dx == 14),
                                tile_position=(64 * R, 32 * cg),
                            )

                ot = o_pool.tile([128, 2, 4, 128], FP16, tag="ot")
                nc.vector.tensor_copy(ot[:, 0, :, :], ps[0][:, :, :])
                nc.scalar.copy(ot[:, 1, :, :], ps[1][:, :, :])
                nc.gpsimd.dma_start(out=out_d[s], in_=ot[:])

    nc.compile()
    return nc


def _host_prep(patches_pairs: np.ndarray, kernels_pairs: np.ndarray):
    """[NJ,128,128] f32, [NJ,15,15] f32 -> xprep [S,2,46,4,4,142] fp16,
    tprep [S,2,46,4,15,32] fp16 laid out for the 8 PE tile slots.
    Job (s,R,cg) = jobs[8s + 4R + cg]."""
    NJ = patches_pairs.shape[0]
    S = NJ // 8
    Xp = np.zeros((NJ, 142, 142), np.float16)
    Xp[:, 7:135, 7:135] = patches_pairs.astype(np.float16)
    s0, s1, s2 = Xp.strides
    W4 = np.lib.stride_tricks.as_strided(Xp, (NJ, 4, 46, 142), (s0, 32 * s1, s1, s2))
    jv = W4.reshape(S, 2, 4, 4, 46, 142)  # [s, R, cg, b, i, x]
    xprep = np.zeros((S, 2, 64, 4, 4, 142), np.float16)
    xprep[:, :, :46] = jv.transpose(0, 1, 4, 2, 3, 5)  # (s, R, i, cg, b, x)
    xprep = xprep.reshape(S, 128, 4, 4, 142)

    Wf = kernels_pairs[:, ::-1, ::-1].astype(np.float16)  # [NJ, dy, dx]
    H = np.zeros((NJ, 78, 15), np.float16)
    H[:, 32:47, :] = Wf
    h0, h1, h2 = H.strides
    B = np.lib.stride_tricks.as_strided(H[:, 32:, :], (NJ, 32, 46, 15), (h0, -h1, h1, h2))
    T = np.ascontiguousarray(B.transpose(0, 3, 2, 1))  # [NJ, dx, i, jj] = Wf[i-jj, dx]
    Tj = T.reshape(S, 2, 4, 15, 46, 32)  # [s, R, cg, dx, i, jj]
    tprep = np.zeros((S, 2, 64, 4, 15, 32), np.float16)
    tprep[:, :, :46] = Tj.transpose(0, 1, 4, 2, 3, 5)  # (s, R, i, cg, dx, jj)
    tprep = tprep.reshape(S, 128, 4, 15, 32)
    return xprep, tprep


def _reassemble(res: np.ndarray, NJ: int) -> np.ndarray:
    """res [S, 128p, 2R, 4b, 128x] fp16 -> [NJ, 128, 128] f32."""
    S = res.shape[0]
    r6 = res.reshape(S, 4, 32, 2, 4, 128)  # (s, cg, i, R, b, x)
    return r6.transpose(0, 3, 1, 4, 2, 5).reshape(NJ, 128, 128).astype(np.float32)


def kernel(patches, kernels, kernel_size, patch_size, fft_size, _collect_results=None):
    """Full inputs in, full output out. Shards BN across 8 cores."""
    from concourse.bass_utils import run_bass_kernel_spmd

    patches = np.asarray(patches)
    kernels = np.asarray(kernels)
    assert patches.shape == (_BN, _C, _P, _P), patches.shape
    assert kernels.shape == (_BN, _C, _K, _K), kernels.shape

    if "nc" not in _nc_cache:
        _nc_cache["nc"] = _build_nc(_SETS_PER_CORE)
    nc = _nc_cache["nc"]

    bn_per_core = _BN // _N_CORES
    in_maps = []
    for core in range(_N_CORES):
        sl = slice(core * bn_per_core, (core + 1) * bn_per_core)
        pp = patches[sl].reshape(-1, _P, _P)
        kp = kernels[sl].reshape(-1, _K, _K)
        xprep, tprep = _host_prep(pp, kp)
        in_maps.append({"xprep": xprep, "tprep": tprep})

    res = run_bass_kernel_spmd(nc, in_maps, core_ids=list(range(_N_CORES)))
    if _collect_results is not None:
        _collect_results.append(res)

    out = np.empty((_BN, _C, _P, _P), dtype=np.float32)
    for core in range(_N_CORES):
        sl = slice(core * bn_per_core, (core + 1) * bn_per_core)
        out[sl] = _reassemble(res.results[core]["out"], _PAIRS_PER_CORE).reshape(
            bn_per_core, _C, _P, _P
        )
    return out
